# revision 17
# baseline (speedup 1.0000x reference)
"""Trainium2 Bass kernel for nn_MultiScaleGeometricAttention.

Reference semantics (ages=0 => attention_scale = 0.05):
    eff_t[n] = (|temperature[n]| + 0.1) * 0.05
    q[r, n]  = ||x_r||^2 + ||p_n||^2 - 2 * (x_r . p_n)
    d = sqrt(q);   w = exp(-d / eff_t)
    out = (w @ values) / (w @ 1 + 1e-8)
(per-row normalization commutes with the GEMM, so it is applied after)

Sharding: data-parallel over flattened B*T rows; 2048 rows per core on 8 cores.
positions/values/temperature are replicated.

Per-core device pipeline (layout S^T: n on partitions, rows on free axis):
    GEMM1 (PE, bf16):  psq[n, r] = (-2 p)^T @ x        (f32 PSUM accum over D)
    DVE:               q = (psq + p2[n]) + x2[r]       (one scalar_tensor_tensor)
    ACT:               d = sqrt(q)  (batched);  w[n,r] = exp(nit[n] * d)
    GEMM2 (PE, bf16):  o[r, d] = w^T.T @ v;  s[r] = w^T.T @ 1
    DVE:               out = o * (1 / (s + 1e-8))
"""

import sys

if "/opt/trn_rl_repo" not in sys.path:
    sys.path.insert(0, "/opt/trn_rl_repo")

import numpy as np
import ml_dtypes

P = 128
CHUNK = 512  # row-columns of S^T processed per chunk (PSUM free dim)
GROUP = 8    # n-tiles per ACT batch group

N_CORES = 8


def build_program(R=2048, N=4096, D=512):
    import concourse.bass as bass
    import concourse.mybir as mybir
    import concourse.tile as tile
    from concourse import bacc

    f32 = mybir.dt.float32
    bf16 = mybir.dt.bfloat16
    Alu = mybir.AluOpType
    Act = mybir.ActivationFunctionType

    KT = D // P      # contraction tiles for GEMM1
    NT = N // P      # n tiles
    CH = R // CHUNK  # chunks
    BTT = CHUNK // P # row tiles per chunk
    NG = NT // GROUP # groups per chunk

    nc = bacc.Bacc()
    xT = nc.declare_dram_parameter("xT", [D, R], bf16, isOutput=False)
    posTm2 = nc.declare_dram_parameter("posTm2", [D, N], bf16, isOutput=False)
    v = nc.declare_dram_parameter("v", [N, D], bf16, isOutput=False)
    # aux: [:, :R] = x2 broadcast, [:, R:R+NT] = p2, [:, R+NT:R+2NT] = -1/eff_t
    aux = nc.declare_dram_parameter("aux", [P, R + 2 * NT], f32, isOutput=False)
    out = nc.declare_dram_parameter("out", [R, D], f32, isOutput=True)

    with tile.TileContext(nc) as tc:
        with (
            tc.tile_pool(name="singles", bufs=1) as singles,
            tc.tile_pool(name="xt", bufs=2) as xt_pool,
            tc.tile_pool(name="q", bufs=3) as q_pool,
            tc.tile_pool(name="w", bufs=3) as w_pool,
            tc.tile_pool(name="o", bufs=4) as o_pool,
            tc.tile_pool(name="sr", bufs=2) as sr_pool,
            tc.tile_pool(name="psq", bufs=2, space="PSUM") as psq_pool,
            tc.tile_pool(name="pso", bufs=1, space="PSUM") as pso_pool,
            tc.tile_pool(name="psr", bufs=1, space="PSUM") as psr_pool,
            tc.tile_pool(name="rpt", bufs=1, space="PSUM") as rpt_pool,
        ):
            posT_sb = singles.tile([P, KT, N], bf16)
            nc.sync.dma_start(
                out=posT_sb, in_=posTm2[:, :].rearrange("(kt p) n -> p kt n", p=P)
            )
            v_sb = singles.tile([P, NT, D], bf16)
            nc.sync.dma_start(
                out=v_sb, in_=v[:, :].rearrange("(nt p) d -> p nt d", p=P)
            )
            aux_sb = singles.tile([P, R + 2 * NT], f32)
            nc.sync.dma_start(out=aux_sb, in_=aux[:, :])
            x2b_sb = aux_sb[:, :R]
            p2_sb = aux_sb[:, R : R + NT]
            nit_sb = aux_sb[:, R + NT : R + 2 * NT]
            ones_sb = singles.tile([P, 1], bf16)
            nc.vector.memset(ones_sb, 1.0)
            one1_sb = singles.tile([1, 1], f32)
            nc.vector.memset(one1_sb, 1.0)
            eps_sb = singles.tile([1, CHUNK], f32)
            nc.vector.memset(eps_sb, 1e-8)

            xT_r = xT[:, :].rearrange("(kt p) r -> p kt r", p=P)

            for c in range(CH):
                c0 = c * CHUNK
                xt = xt_pool.tile([P, KT, CHUNK], bf16, tag="xt")
                nc.sync.dma_start(out=xt, in_=xT_r[:, :, c0 : c0 + CHUNK])

                pso_tiles = [
                    pso_pool.tile([P, D], f32, tag=f"pso{i}", name=f"pso{i}")
                    for i in range(BTT)
                ]
                srow_ps = psr_pool.tile([1, CHUNK], f32, tag="psrow")
                w_tiles = [None] * NG

                def emit_g1(g, xt=xt, c0=c0, w_tiles=w_tiles):
                    q_g = q_pool.tile([P, GROUP, CHUNK], bf16, tag="q")
                    for jj in range(GROUP):
                        j = g * GROUP + jj
                        psq = psq_pool.tile([P, CHUNK], f32, tag="psq")
                        for k in range(KT):
                            nc.tensor.matmul(
                                psq,
                                posT_sb[:, k, j * P : (j + 1) * P],
                                xt[:, k, :],
                                start=(k == 0),
                                stop=(k == KT - 1),
                            )
                        # q = psq + x2[r]
                        nc.vector.tensor_tensor(
                            q_g[:, jj, :],
                            psq,
                            x2b_sb[:, c0 : c0 + CHUNK],
                            Alu.add,
                        )
                    # d = sqrt(q + p2[n]) (batched per func to avoid ACT
                    # table swaps), then w = exp(-d / eff_t)
                    for jj in range(GROUP):
                        j = g * GROUP + jj
                        nc.scalar.activation(
                            out=q_g[:, jj, :],
                            in_=q_g[:, jj, :],
                            func=Act.Sqrt,
                            bias=p2_sb[:, j : j + 1],
                        )
                    w_g = w_pool.tile([P, GROUP, CHUNK], bf16, tag="w")
                    for jj in range(GROUP):
                        j = g * GROUP + jj
                        nc.scalar.activation(
                            out=w_g[:, jj, :],
                            in_=q_g[:, jj, :],
                            func=Act.Exp,
                            scale=nit_sb[:, j : j + 1],
                        )
                    w_tiles[g] = w_g

                def emit_g2(g, pso_tiles=pso_tiles, srow_ps=srow_ps, w_tiles=w_tiles):
                    w_g = w_tiles[g]
                    for jj in range(GROUP):
                        j = g * GROUP + jj
                        first = j == 0
                        last = j == NT - 1
                        # row sums: s_row[1, r] += ones.T @ w_g[:, jj, :]
                        nc.tensor.matmul(
                            srow_ps,
                            ones_sb,
                            w_g[:, jj, :],
                            start=first,
                            stop=last,
                        )
                        for i in range(BTT):
                            lhsT = w_g[:, jj, i * P : (i + 1) * P]
                            nc.tensor.matmul(
                                pso_tiles[i],
                                lhsT,
                                v_sb[:, j, :],
                                start=first,
                                stop=last,
                            )

                for g in range(NG):
                    emit_g1(g)
                    if g >= 1:
                        emit_g2(g - 1)
                emit_g2(NG - 1)

                # normalize: out_i = pso_i * (1 / (s_i + 1e-8))
                s_sb = sr_pool.tile([1, CHUNK], f32, tag="s")
                nc.vector.tensor_tensor(s_sb, srow_ps, eps_sb, Alu.add)
                nc.vector.reciprocal(out=s_sb, in_=s_sb)
                # transpose r_row [1, CHUNK] -> [P, BTT] via K=1 matmuls
                # (single group: start clears the bank, later MMs overwrite
                # their own fresh columns)
                rpt_ps = rpt_pool.tile([P, BTT], f32, tag="rpt")
                for i in range(BTT):
                    nc.tensor.matmul(
                        rpt_ps[:, i : i + 1],
                        s_sb[0:1, i * P : (i + 1) * P],
                        one1_sb,
                        start=(i == 0),
                        stop=(i == BTT - 1),
                    )
                r_sb = sr_pool.tile([P, BTT], f32, tag="r")
                nc.vector.tensor_copy(out=r_sb, in_=rpt_ps)
                for i in range(BTT):
                    o_sb = o_pool.tile([P, D], f32, tag="o")
                    nc.vector.tensor_tensor(
                        o_sb,
                        pso_tiles[i],
                        r_sb[:, i : i + 1].to_broadcast([P, D]),
                        Alu.mult,
                    )
                    nc.sync.dma_start(
                        out=out[c0 + i * P : c0 + (i + 1) * P, :], in_=o_sb
                    )
    nc.finalize()
    return nc


def prepare_in_maps(x, positions, values, temperature, n_cores=N_CORES):
    bf16 = ml_dtypes.bfloat16
    x = np.asarray(x, np.float32)
    positions = np.asarray(positions, np.float32)
    values = np.asarray(values, np.float32)
    temperature = np.asarray(temperature, np.float32)

    B, T, D = x.shape
    N = positions.shape[0]
    xf = x.reshape(-1, D)
    R = xf.shape[0] // n_cores

    # attention scale with ages=0: 0.05 + 0.95 * (1 - exp(0)) = 0.05
    eff_t = (np.abs(temperature) + 0.1) * np.float32(0.05)
    nit_full = (-1.0 / eff_t).astype(np.float32)        # [N]
    p2_full = (positions * positions).sum(1).astype(np.float32)  # [N]
    NT = N // P
    p2_pt = np.ascontiguousarray(p2_full.reshape(NT, P).T)
    nit_pt = np.ascontiguousarray(nit_full.reshape(NT, P).T)
    posTm2 = np.ascontiguousarray((-2.0 * positions).T).astype(bf16)
    v_bf = np.ascontiguousarray(values).astype(bf16)

    maps = []
    for ci in range(n_cores):
        xc = xf[ci * R : (ci + 1) * R]
        x2c = (xc * xc).sum(1, dtype=np.float32)
        aux = np.empty((P, R + 2 * NT), np.float32)
        aux[:, :R] = x2c[None, :]
        aux[:, R : R + NT] = p2_pt
        aux[:, R + NT : R + 2 * NT] = nit_pt
        maps.append(
            dict(
                xT=np.ascontiguousarray(xc.T).astype(bf16),
                posTm2=posTm2,
                v=v_bf,
                aux=aux,
            )
        )
    return maps


_prog_cache = {}


def get_program():
    if "nc" not in _prog_cache:
        _prog_cache["nc"] = build_program()
    return _prog_cache["nc"]


def kernel(x, positions, values, temperature):
    from concourse.bass_utils import run_bass_kernel_spmd

    maps = prepare_in_maps(x, positions, values, temperature)
    nc = get_program()
    res = run_bass_kernel_spmd(nc, maps, list(range(N_CORES)))
    B, T, D = np.asarray(x).shape
    out = np.concatenate(
        [np.asarray(res.results[i]["out"]) for i in range(N_CORES)], axis=0
    )
    return np.ascontiguousarray(out.reshape(B, T, D)).astype(np.float32)


# revision 22
# speedup vs baseline: 1.0501x; 1.0501x over previous
"""Trainium2 Bass kernel for nn_MultiScaleGeometricAttention.

Reference semantics (ages=0 => attention_scale = 0.05):
    eff_t[n] = (|temperature[n]| + 0.1) * 0.05
    q[r, n]  = ||x_r||^2 + ||p_n||^2 - 2 * (x_r . p_n)
    d = sqrt(q);   w = exp(-d / eff_t)
    out = (w @ values) / (w @ 1 + 1e-8)
(per-row normalization commutes with the GEMM, so it is applied after)

Sharding: data-parallel over flattened B*T rows; 2048 rows per core on 8 cores.
positions/values/temperature are replicated.

Per-core device pipeline (layout S^T: n on partitions, rows on free axis):
    GEMM1 (PE, bf16):  psq[n, r] = (-2 p)^T @ x        (f32 PSUM accum over D)
    DVE:               q = (psq + p2[n]) + x2[r]       (one scalar_tensor_tensor)
    ACT:               d = sqrt(q)  (batched);  w[n,r] = exp(nit[n] * d)
    GEMM2 (PE, bf16):  o[r, d] = w^T.T @ v;  s[r] = w^T.T @ 1
    DVE:               out = o * (1 / (s + 1e-8))
"""

import sys

if "/opt/trn_rl_repo" not in sys.path:
    sys.path.insert(0, "/opt/trn_rl_repo")

import numpy as np
import ml_dtypes

P = 128
CHUNK = 512  # row-columns of S^T processed per chunk (PSUM free dim)
GROUP = 16   # n-tiles per ACT batch group

N_CORES = 8


def build_program(R=2048, N=4096, D=512):
    import concourse.bass as bass
    import concourse.mybir as mybir
    import concourse.tile as tile
    from concourse import bacc

    f32 = mybir.dt.float32
    bf16 = mybir.dt.bfloat16
    Alu = mybir.AluOpType
    Act = mybir.ActivationFunctionType

    KT = D // P      # contraction tiles for GEMM1
    NT = N // P      # n tiles
    CH = R // CHUNK  # chunks
    BTT = CHUNK // P # row tiles per chunk
    GRP = min(GROUP, NT)
    NG = NT // GRP   # groups per chunk

    nc = bacc.Bacc()
    xT = nc.declare_dram_parameter("xT", [D, R], bf16, isOutput=False)
    posTm2 = nc.declare_dram_parameter("posTm2", [D, N], bf16, isOutput=False)
    v = nc.declare_dram_parameter("v", [N, D], bf16, isOutput=False)
    # aux: [:, :R] = x2 broadcast, [:, R:R+NT] = p2, [:, R+NT:R+2NT] = -1/eff_t
    aux = nc.declare_dram_parameter("aux", [P, R + 2 * NT], f32, isOutput=False)
    out = nc.declare_dram_parameter("out", [R, D], f32, isOutput=True)

    with tile.TileContext(nc) as tc:
        with (
            tc.tile_pool(name="singles", bufs=1) as singles,
            tc.tile_pool(name="xt", bufs=2) as xt_pool,
            tc.tile_pool(name="q", bufs=2) as q_pool,
            tc.tile_pool(name="w", bufs=2) as w_pool,
            tc.tile_pool(name="o", bufs=4) as o_pool,
            tc.tile_pool(name="sr", bufs=2) as sr_pool,
            tc.tile_pool(name="psq", bufs=2, space="PSUM") as psq_pool,
            tc.tile_pool(name="pso", bufs=1, space="PSUM") as pso_pool,
            tc.tile_pool(name="psr", bufs=1, space="PSUM") as psr_pool,
            tc.tile_pool(name="rpt", bufs=1, space="PSUM") as rpt_pool,
        ):
            xT_r = xT[:, :].rearrange("(kt p) r -> p kt r", p=P)
            xts = [None] * CH

            def load_xt(c):
                t = xt_pool.tile([P, KT, CHUNK], bf16, tag="xt", name=f"xt{c}")
                nc.sync.dma_start(
                    out=t, in_=xT_r[:, :, c * CHUNK : (c + 1) * CHUNK]
                )
                xts[c] = t

            # first chunk of x lands first so GEMM1 can start ASAP
            load_xt(0)
            # positions: one tile per k so matmuls start after the first slice
            posTm2_r = posTm2[:, :].rearrange("(kt p) n -> kt p n", p=P)
            posT_tiles = []
            for k in range(KT):
                pt = singles.tile([P, N], bf16, name=f"posT{k}")
                nc.sync.dma_start(out=pt, in_=posTm2_r[k])
                posT_tiles.append(pt)
            # aux + values go on the gpsimd queue (not needed until later)
            aux_sb = singles.tile([P, R + 2 * NT], f32)
            nc.gpsimd.dma_start(out=aux_sb, in_=aux[:, :])
            x2b_sb = aux_sb[:, :R]
            p2_sb = aux_sb[:, R : R + NT]
            nit_sb = aux_sb[:, R + NT : R + 2 * NT]
            v_sb = singles.tile([P, NT, D], bf16)
            v_r = v[:, :].rearrange("(vh nt p) d -> vh p nt d", p=P, vh=4)
            for h in range(4):
                nc.gpsimd.dma_start(
                    out=v_sb[:, h * (NT // 4) : (h + 1) * (NT // 4), :], in_=v_r[h]
                )
            ones_sb = singles.tile([P, 1], bf16)
            nc.vector.memset(ones_sb, 1.0)
            one1_sb = singles.tile([1, 1], f32)
            nc.vector.memset(one1_sb, 1.0)
            eps_sb = singles.tile([1, CHUNK], f32)
            nc.vector.memset(eps_sb, 1e-8)

            for c in range(CH):
                c0 = c * CHUNK
                if c + 1 < CH:
                    load_xt(c + 1)
                xt = xts[c]

                pso_tiles = [
                    pso_pool.tile([P, D], f32, tag=f"pso{i}", name=f"pso{i}")
                    for i in range(BTT)
                ]
                srow_ps = psr_pool.tile([1, CHUNK], f32, tag="psrow")
                w_tiles = [None] * NG

                def emit_g1(g, xt=xt, c0=c0, w_tiles=w_tiles):
                    q_g = q_pool.tile([P, GRP, CHUNK], bf16, tag="q")
                    for jj in range(GRP):
                        j = g * GRP + jj
                        psq = psq_pool.tile([P, CHUNK], f32, tag="psq")
                        for k in range(KT):
                            nc.tensor.matmul(
                                psq,
                                posT_tiles[k][:, j * P : (j + 1) * P],
                                xt[:, k, :],
                                start=(k == 0),
                                stop=(k == KT - 1),
                            )
                        # q = psq + x2[r]
                        nc.vector.tensor_tensor(
                            q_g[:, jj, :],
                            psq,
                            x2b_sb[:, c0 : c0 + CHUNK],
                            Alu.add,
                        )
                    # d = sqrt(q + p2[n]) (batched per func to avoid ACT
                    # table swaps), then w = exp(-d / eff_t)
                    for jj in range(GRP):
                        j = g * GRP + jj
                        nc.scalar.activation(
                            out=q_g[:, jj, :],
                            in_=q_g[:, jj, :],
                            func=Act.Sqrt,
                            bias=p2_sb[:, j : j + 1],
                        )
                    w_g = w_pool.tile([P, GRP, CHUNK], bf16, tag="w")
                    for jj in range(GRP):
                        j = g * GRP + jj
                        nc.scalar.activation(
                            out=w_g[:, jj, :],
                            in_=q_g[:, jj, :],
                            func=Act.Exp,
                            scale=nit_sb[:, j : j + 1],
                        )
                    w_tiles[g] = w_g

                def emit_g2(g, pso_tiles=pso_tiles, srow_ps=srow_ps, w_tiles=w_tiles):
                    w_g = w_tiles[g]
                    for jj in range(GRP):
                        j = g * GRP + jj
                        first = j == 0
                        last = j == NT - 1
                        # row sums: s_row[1, r] += ones.T @ w_g[:, jj, :]
                        nc.tensor.matmul(
                            srow_ps,
                            ones_sb,
                            w_g[:, jj, :],
                            start=first,
                            stop=last,
                        )
                        for i in range(BTT):
                            lhsT = w_g[:, jj, i * P : (i + 1) * P]
                            nc.tensor.matmul(
                                pso_tiles[i],
                                lhsT,
                                v_sb[:, j, :],
                                start=first,
                                stop=last,
                            )

                for g in range(NG):
                    emit_g1(g)
                    if g >= 1:
                        emit_g2(g - 1)
                emit_g2(NG - 1)

                # normalize: out_i = pso_i * (1 / (s_i + 1e-8))
                s_sb = sr_pool.tile([1, CHUNK], f32, tag="s")
                nc.vector.tensor_tensor(s_sb, srow_ps, eps_sb, Alu.add)
                # transpose (s + eps) [1, CHUNK] -> [P, BTT] via K=1 matmuls
                # (single group: start clears the bank, later MMs overwrite
                # their own fresh columns)
                rpt_ps = rpt_pool.tile([P, BTT], f32, tag="rpt")
                for i in range(BTT):
                    nc.tensor.matmul(
                        rpt_ps[:, i : i + 1],
                        s_sb[0:1, i * P : (i + 1) * P],
                        one1_sb,
                        start=(i == 0),
                        stop=(i == BTT - 1),
                    )
                # reciprocal on 128 partitions (fast) instead of on [1, CHUNK]
                r_sb = sr_pool.tile([P, BTT], f32, tag="r")
                nc.vector.reciprocal(out=r_sb, in_=rpt_ps)
                for i in range(BTT):
                    o_sb = o_pool.tile([P, D], f32, tag="o")
                    nc.vector.tensor_tensor(
                        o_sb,
                        pso_tiles[i],
                        r_sb[:, i : i + 1].to_broadcast([P, D]),
                        Alu.mult,
                    )
                    nc.gpsimd.dma_start(
                        out=out[c0 + i * P : c0 + (i + 1) * P, :], in_=o_sb
                    )
    nc.finalize()
    return nc


def prepare_in_maps(x, positions, values, temperature, n_cores=N_CORES):
    bf16 = ml_dtypes.bfloat16
    x = np.asarray(x, np.float32)
    positions = np.asarray(positions, np.float32)
    values = np.asarray(values, np.float32)
    temperature = np.asarray(temperature, np.float32)

    B, T, D = x.shape
    N = positions.shape[0]
    xf = x.reshape(-1, D)
    R = xf.shape[0] // n_cores

    # attention scale with ages=0: 0.05 + 0.95 * (1 - exp(0)) = 0.05
    eff_t = (np.abs(temperature) + 0.1) * np.float32(0.05)
    nit_full = (-1.0 / eff_t).astype(np.float32)        # [N]
    p2_full = (positions * positions).sum(1).astype(np.float32)  # [N]
    NT = N // P
    p2_pt = np.ascontiguousarray(p2_full.reshape(NT, P).T)
    nit_pt = np.ascontiguousarray(nit_full.reshape(NT, P).T)
    posTm2 = np.ascontiguousarray((-2.0 * positions).T).astype(bf16)
    v_bf = np.ascontiguousarray(values).astype(bf16)

    maps = []
    for ci in range(n_cores):
        xc = xf[ci * R : (ci + 1) * R]
        x2c = (xc * xc).sum(1, dtype=np.float32)
        aux = np.empty((P, R + 2 * NT), np.float32)
        aux[:, :R] = x2c[None, :]
        aux[:, R : R + NT] = p2_pt
        aux[:, R + NT : R + 2 * NT] = nit_pt
        maps.append(
            dict(
                xT=np.ascontiguousarray(xc.T).astype(bf16),
                posTm2=posTm2,
                v=v_bf,
                aux=aux,
            )
        )
    return maps


_prog_cache = {}


def get_program():
    if "nc" not in _prog_cache:
        _prog_cache["nc"] = build_program()
    return _prog_cache["nc"]


def kernel(x, positions, values, temperature):
    from concourse.bass_utils import run_bass_kernel_spmd

    maps = prepare_in_maps(x, positions, values, temperature)
    nc = get_program()
    res = run_bass_kernel_spmd(nc, maps, list(range(N_CORES)))
    B, T, D = np.asarray(x).shape
    out = np.concatenate(
        [np.asarray(res.results[i]["out"]) for i in range(N_CORES)], axis=0
    )
    return np.ascontiguousarray(out.reshape(B, T, D)).astype(np.float32)


# revision 26
# speedup vs baseline: 1.0847x; 1.0329x over previous
"""Trainium2 Bass kernel for nn_MultiScaleGeometricAttention.

Reference semantics (ages=0 => attention_scale = 0.05):
    eff_t[n] = (|temperature[n]| + 0.1) * 0.05
    q[r, n]  = ||x_r||^2 + ||p_n||^2 - 2 * (x_r . p_n)
    d = sqrt(q);   w = exp(-d / eff_t)
    out = (w @ values) / (w @ 1 + 1e-8)
(per-row normalization commutes with the GEMM, so it is applied after)

Sharding: data-parallel over flattened B*T rows; 2048 rows per core on 8 cores.
positions/values/temperature are replicated.

Per-core device pipeline (layout S^T: n on partitions, rows on free axis):
    GEMM1 (PE, bf16):  psq[n, r] = (-2 p)^T @ x        (f32 PSUM accum over D)
    DVE:               q = (psq + p2[n]) + x2[r]       (one scalar_tensor_tensor)
    ACT:               d = sqrt(q)  (batched);  w[n,r] = exp(nit[n] * d)
    GEMM2 (PE, bf16):  o[r, d] = w^T.T @ v;  s[r] = w^T.T @ 1
    DVE:               out = o * (1 / (s + 1e-8))
"""

import sys

if "/opt/trn_rl_repo" not in sys.path:
    sys.path.insert(0, "/opt/trn_rl_repo")

import numpy as np
import ml_dtypes

P = 128
CHUNK = 512  # row-columns of S^T processed per chunk (PSUM free dim)
GROUP = 16   # n-tiles per ACT batch group

N_CORES = 8


def build_program(R=2048, N=4096, D=512):
    import concourse.bass as bass
    import concourse.mybir as mybir
    import concourse.tile as tile
    from concourse import bacc

    f32 = mybir.dt.float32
    bf16 = mybir.dt.bfloat16
    Alu = mybir.AluOpType
    Act = mybir.ActivationFunctionType

    KT = D // P      # contraction tiles for GEMM1
    NT = N // P      # n tiles
    CH = R // CHUNK  # chunks
    BTT = CHUNK // P # row tiles per chunk
    GRP = min(GROUP, NT)
    NG = NT // GRP   # groups per chunk

    nc = bacc.Bacc()
    xT = nc.declare_dram_parameter("xT", [D, R], bf16, isOutput=False)
    posTm2 = nc.declare_dram_parameter("posTm2", [D, N], bf16, isOutput=False)
    v = nc.declare_dram_parameter("v", [N, D], bf16, isOutput=False)
    # aux: [:, :R] = x2 broadcast, [:, R:R+NT] = p2, [:, R+NT:R+2NT] = -1/eff_t
    aux = nc.declare_dram_parameter("aux", [P, R + 2 * NT], f32, isOutput=False)
    out = nc.declare_dram_parameter("out", [R, D], f32, isOutput=True)

    with tile.TileContext(nc) as tc:
        with (
            tc.tile_pool(name="singles", bufs=1) as singles,
            tc.tile_pool(name="xt", bufs=2) as xt_pool,
            tc.tile_pool(name="q", bufs=2) as q_pool,
            tc.tile_pool(name="w", bufs=2) as w_pool,
            tc.tile_pool(name="o", bufs=4) as o_pool,
            tc.tile_pool(name="sr", bufs=2) as sr_pool,
            tc.tile_pool(name="psq", bufs=2, space="PSUM") as psq_pool,
            tc.tile_pool(name="pso", bufs=1, space="PSUM") as pso_pool,
            tc.tile_pool(name="psr", bufs=1, space="PSUM") as psr_pool,
            tc.tile_pool(name="rpt", bufs=1, space="PSUM") as rpt_pool,
        ):
            xT_r = xT[:, :].rearrange("(kt p) r -> p kt r", p=P)
            xts = [None] * CH

            def load_xt(c):
                t = xt_pool.tile([P, KT, CHUNK], bf16, tag="xt", name=f"xt{c}")
                nc.sync.dma_start(
                    out=t, in_=xT_r[:, :, c * CHUNK : (c + 1) * CHUNK]
                )
                xts[c] = t

            # first chunk of x lands first so GEMM1 can start ASAP
            load_xt(0)
            # positions: one tile per k, spread across DMA queues so all
            # four k-slices arrive in parallel
            posTm2_r = posTm2[:, :].rearrange("(kt p) n -> kt p n", p=P)
            posT_tiles = []
            dma_engines = [nc.sync, nc.scalar, nc.gpsimd, nc.scalar]
            for k in range(KT):
                pt = singles.tile([P, N], bf16, name=f"posT{k}")
                dma_engines[k % len(dma_engines)].dma_start(out=pt, in_=posTm2_r[k])
                posT_tiles.append(pt)
            # aux + values spread over gpsimd/sync (not needed until GEMM2)
            aux_sb = singles.tile([P, R + 2 * NT], f32)
            nc.gpsimd.dma_start(out=aux_sb, in_=aux[:, :])
            x2b_sb = aux_sb[:, :R]
            p2_sb = aux_sb[:, R : R + NT]
            nit_sb = aux_sb[:, R + NT : R + 2 * NT]
            v_sb = singles.tile([P, NT, D], bf16)
            v_r = v[:, :].rearrange("(vh nt p) d -> vh p nt d", p=P, vh=4)
            v_engines = [nc.gpsimd, nc.gpsimd, nc.sync, nc.sync]
            for h in range(4):
                v_engines[h].dma_start(
                    out=v_sb[:, h * (NT // 4) : (h + 1) * (NT // 4), :], in_=v_r[h]
                )
            ones_sb = singles.tile([P, 1], bf16)
            nc.vector.memset(ones_sb, 1.0)
            one1_sb = singles.tile([1, 1], f32)
            nc.vector.memset(one1_sb, 1.0)
            eps_sb = singles.tile([1, CHUNK], f32)
            nc.vector.memset(eps_sb, 1e-8)

            pending_drain = [None]
            for c in range(CH):
                c0 = c * CHUNK
                if c + 1 < CH:
                    load_xt(c + 1)
                xt = xts[c]

                pso_tiles = [
                    pso_pool.tile([P, D], f32, tag=f"pso{i}", name=f"pso{i}")
                    for i in range(BTT)
                ]
                srow_ps = psr_pool.tile([1, CHUNK], f32, tag="psrow")
                w_tiles = [None] * NG

                def emit_g1(g, xt=xt, c0=c0, w_tiles=w_tiles):
                    q_g = q_pool.tile([P, GRP, CHUNK], bf16, tag="q")
                    for jj in range(GRP):
                        j = g * GRP + jj
                        psq = psq_pool.tile([P, CHUNK], f32, tag="psq")
                        for k in range(KT):
                            nc.tensor.matmul(
                                psq,
                                posT_tiles[k][:, j * P : (j + 1) * P],
                                xt[:, k, :],
                                start=(k == 0),
                                stop=(k == KT - 1),
                            )
                        # q = psq + x2[r]
                        nc.vector.tensor_tensor(
                            q_g[:, jj, :],
                            psq,
                            x2b_sb[:, c0 : c0 + CHUNK],
                            Alu.add,
                        )
                    # d = sqrt(q + p2[n]) (batched per func to avoid ACT
                    # table swaps), then w = exp(-d / eff_t)
                    for jj in range(GRP):
                        j = g * GRP + jj
                        nc.scalar.activation(
                            out=q_g[:, jj, :],
                            in_=q_g[:, jj, :],
                            func=Act.Sqrt,
                            bias=p2_sb[:, j : j + 1],
                        )
                    w_g = w_pool.tile([P, GRP, CHUNK], bf16, tag="w")
                    for jj in range(GRP):
                        j = g * GRP + jj
                        nc.scalar.activation(
                            out=w_g[:, jj, :],
                            in_=q_g[:, jj, :],
                            func=Act.Exp,
                            scale=nit_sb[:, j : j + 1],
                        )
                    w_tiles[g] = w_g

                def emit_g2(g, pso_tiles=pso_tiles, srow_ps=srow_ps, w_tiles=w_tiles):
                    w_g = w_tiles[g]
                    for jj in range(GRP):
                        j = g * GRP + jj
                        first = j == 0
                        last = j == NT - 1
                        # row sums: s_row[1, r] += ones.T @ w_g[:, jj, :]
                        nc.tensor.matmul(
                            srow_ps,
                            ones_sb,
                            w_g[:, jj, :],
                            start=first,
                            stop=last,
                        )
                        for i in range(BTT):
                            lhsT = w_g[:, jj, i * P : (i + 1) * P]
                            nc.tensor.matmul(
                                pso_tiles[i],
                                lhsT,
                                v_sb[:, j, :],
                                start=first,
                                stop=last,
                            )

                def make_drain(c0=c0, pso_tiles=pso_tiles, srow_ps=srow_ps):
                    def drain():
                        # normalize: out_i = pso_i * (1 / (s_i + 1e-8))
                        s_sb = sr_pool.tile([1, CHUNK], f32, tag="s", name="s_sb")
                        nc.vector.tensor_tensor(s_sb, srow_ps, eps_sb, Alu.add)
                        # transpose (s + eps) [1, CHUNK] -> [P, BTT] via K=1
                        # matmuls (single group: start clears the bank, later
                        # MMs overwrite their own fresh columns)
                        rpt_ps = rpt_pool.tile([P, BTT], f32, tag="rpt", name="rpt")
                        for i in range(BTT):
                            nc.tensor.matmul(
                                rpt_ps[:, i : i + 1],
                                s_sb[0:1, i * P : (i + 1) * P],
                                one1_sb,
                                start=(i == 0),
                                stop=(i == BTT - 1),
                            )
                        # reciprocal on 128 partitions instead of on [1, CHUNK]
                        r_sb = sr_pool.tile([P, BTT], f32, tag="r", name="r_sb")
                        nc.vector.reciprocal(out=r_sb, in_=rpt_ps)
                        for i in range(BTT):
                            o_sb = o_pool.tile([P, D], f32, tag="o", name="o_sb")
                            nc.vector.tensor_tensor(
                                o_sb,
                                pso_tiles[i],
                                r_sb[:, i : i + 1].to_broadcast([P, D]),
                                Alu.mult,
                            )
                            nc.gpsimd.dma_start(
                                out=out[c0 + i * P : c0 + (i + 1) * P, :], in_=o_sb
                            )
                    return drain

                for g in range(NG):
                    emit_g1(g)
                    if g == 0 and pending_drain[0] is not None:
                        # drain the previous chunk while this chunk's first
                        # GEMM1 group keeps the PE busy
                        pending_drain[0]()
                        pending_drain[0] = None
                    if g >= 1:
                        emit_g2(g - 1)
                emit_g2(NG - 1)
                pending_drain[0] = make_drain()
            pending_drain[0]()
    nc.finalize()
    return nc


def prepare_in_maps(x, positions, values, temperature, n_cores=N_CORES):
    bf16 = ml_dtypes.bfloat16
    x = np.asarray(x, np.float32)
    positions = np.asarray(positions, np.float32)
    values = np.asarray(values, np.float32)
    temperature = np.asarray(temperature, np.float32)

    B, T, D = x.shape
    N = positions.shape[0]
    xf = x.reshape(-1, D)
    R = xf.shape[0] // n_cores

    # attention scale with ages=0: 0.05 + 0.95 * (1 - exp(0)) = 0.05
    eff_t = (np.abs(temperature) + 0.1) * np.float32(0.05)
    nit_full = (-1.0 / eff_t).astype(np.float32)        # [N]
    p2_full = (positions * positions).sum(1).astype(np.float32)  # [N]
    NT = N // P
    p2_pt = np.ascontiguousarray(p2_full.reshape(NT, P).T)
    nit_pt = np.ascontiguousarray(nit_full.reshape(NT, P).T)
    posTm2 = np.ascontiguousarray((-2.0 * positions).T).astype(bf16)
    v_bf = np.ascontiguousarray(values).astype(bf16)

    maps = []
    for ci in range(n_cores):
        xc = xf[ci * R : (ci + 1) * R]
        x2c = (xc * xc).sum(1, dtype=np.float32)
        aux = np.empty((P, R + 2 * NT), np.float32)
        aux[:, :R] = x2c[None, :]
        aux[:, R : R + NT] = p2_pt
        aux[:, R + NT : R + 2 * NT] = nit_pt
        maps.append(
            dict(
                xT=np.ascontiguousarray(xc.T).astype(bf16),
                posTm2=posTm2,
                v=v_bf,
                aux=aux,
            )
        )
    return maps


_prog_cache = {}


def get_program():
    if "nc" not in _prog_cache:
        _prog_cache["nc"] = build_program()
    return _prog_cache["nc"]


def kernel(x, positions, values, temperature):
    from concourse.bass_utils import run_bass_kernel_spmd

    maps = prepare_in_maps(x, positions, values, temperature)
    nc = get_program()
    res = run_bass_kernel_spmd(nc, maps, list(range(N_CORES)))
    B, T, D = np.asarray(x).shape
    out = np.concatenate(
        [np.asarray(res.results[i]["out"]) for i in range(N_CORES)], axis=0
    )
    return np.ascontiguousarray(out.reshape(B, T, D)).astype(np.float32)


# revision 27
# speedup vs baseline: 1.1018x; 1.0158x over previous
"""Trainium2 Bass kernel for nn_MultiScaleGeometricAttention.

Reference semantics (ages=0 => attention_scale = 0.05):
    eff_t[n] = (|temperature[n]| + 0.1) * 0.05
    q[r, n]  = ||x_r||^2 + ||p_n||^2 - 2 * (x_r . p_n)
    d = sqrt(q);   w = exp(-d / eff_t)
    out = (w @ values) / (w @ 1 + 1e-8)
(per-row normalization commutes with the GEMM, so it is applied after)

Sharding: data-parallel over flattened B*T rows; 2048 rows per core on 8 cores.
positions/values/temperature are replicated.

Per-core device pipeline (layout S^T: n on partitions, rows on free axis):
    GEMM1 (PE, bf16):  psq[n, r] = (-2 p)^T @ x        (f32 PSUM accum over D)
    DVE:               q = (psq + p2[n]) + x2[r]       (one scalar_tensor_tensor)
    ACT:               d = sqrt(q)  (batched);  w[n,r] = exp(nit[n] * d)
    GEMM2 (PE, bf16):  o[r, d] = w^T.T @ v;  s[r] = w^T.T @ 1
    DVE:               out = o * (1 / (s + 1e-8))
"""

import sys

if "/opt/trn_rl_repo" not in sys.path:
    sys.path.insert(0, "/opt/trn_rl_repo")

import numpy as np
import ml_dtypes

P = 128
CHUNK = 512  # row-columns of S^T processed per chunk (PSUM free dim)
GROUP = 16   # n-tiles per ACT batch group

N_CORES = 8


def build_program(R=2048, N=4096, D=512):
    import concourse.bass as bass
    import concourse.mybir as mybir
    import concourse.tile as tile
    from concourse import bacc

    f32 = mybir.dt.float32
    bf16 = mybir.dt.bfloat16
    Alu = mybir.AluOpType
    Act = mybir.ActivationFunctionType

    KT = D // P      # contraction tiles for GEMM1
    NT = N // P      # n tiles
    CH = R // CHUNK  # chunks
    BTT = CHUNK // P # row tiles per chunk
    GRP = min(GROUP, NT)
    NG = NT // GRP   # groups per chunk

    nc = bacc.Bacc()
    xT = nc.declare_dram_parameter("xT", [D, R], bf16, isOutput=False)
    posTm2 = nc.declare_dram_parameter("posTm2", [D, N], bf16, isOutput=False)
    v = nc.declare_dram_parameter("v", [N, D], bf16, isOutput=False)
    # aux: [:, :R] = x2 broadcast, [:, R:R+NT] = p2, [:, R+NT:R+2NT] = -1/eff_t
    aux = nc.declare_dram_parameter("aux", [P, R + 2 * NT], f32, isOutput=False)
    out = nc.declare_dram_parameter("out", [R, D], f32, isOutput=True)

    with tile.TileContext(nc) as tc:
        with (
            tc.tile_pool(name="singles", bufs=1) as singles,
            tc.tile_pool(name="xt", bufs=2) as xt_pool,
            tc.tile_pool(name="q", bufs=2) as q_pool,
            tc.tile_pool(name="w", bufs=2) as w_pool,
            tc.tile_pool(name="o", bufs=4) as o_pool,
            tc.tile_pool(name="sr", bufs=2) as sr_pool,
            tc.tile_pool(name="psq", bufs=2, space="PSUM") as psq_pool,
            tc.tile_pool(name="pso", bufs=1, space="PSUM") as pso_pool,
            tc.tile_pool(name="psr", bufs=1, space="PSUM") as psr_pool,
            tc.tile_pool(name="rpt", bufs=1, space="PSUM") as rpt_pool,
        ):
            xT_r = xT[:, :].rearrange("(kt p) r -> p kt r", p=P)
            xts = [None] * CH

            def load_xt(c):
                t = xt_pool.tile([P, KT, CHUNK], bf16, tag="xt", name=f"xt{c}")
                nc.sync.dma_start(
                    out=t, in_=xT_r[:, :, c * CHUNK : (c + 1) * CHUNK]
                )
                xts[c] = t

            # first chunk of x lands first so GEMM1 can start ASAP
            load_xt(0)
            # positions: one tile per k, spread across DMA queues so all
            # four k-slices arrive in parallel
            posTm2_r = posTm2[:, :].rearrange("(kt p) n -> kt p n", p=P)
            posT_tiles = []
            dma_engines = [nc.sync, nc.gpsimd, nc.sync, nc.gpsimd]
            for k in range(KT):
                pt = singles.tile([P, N], bf16, name=f"posT{k}")
                dma_engines[k % len(dma_engines)].dma_start(out=pt, in_=posTm2_r[k])
                posT_tiles.append(pt)
            # aux + values spread over gpsimd/sync (not needed until GEMM2)
            aux_sb = singles.tile([P, R + 2 * NT], f32)
            nc.gpsimd.dma_start(out=aux_sb, in_=aux[:, :])
            x2b_sb = aux_sb[:, :R]
            p2_sb = aux_sb[:, R : R + NT]
            nit_sb = aux_sb[:, R + NT : R + 2 * NT]
            v_sb = singles.tile([P, NT, D], bf16)
            v_r = v[:, :].rearrange("(vh nt p) d -> vh p nt d", p=P, vh=4)
            v_engines = [nc.gpsimd, nc.gpsimd, nc.sync, nc.sync]
            for h in range(4):
                v_engines[h].dma_start(
                    out=v_sb[:, h * (NT // 4) : (h + 1) * (NT // 4), :], in_=v_r[h]
                )
            ones_sb = singles.tile([P, 1], bf16)
            nc.vector.memset(ones_sb, 1.0)
            one1_sb = singles.tile([1, 1], f32)
            nc.vector.memset(one1_sb, 1.0)
            eps_sb = singles.tile([1, CHUNK], f32)
            nc.vector.memset(eps_sb, 1e-8)

            pending_drain = [None]
            for c in range(CH):
                c0 = c * CHUNK
                if c + 1 < CH:
                    load_xt(c + 1)
                xt = xts[c]

                pso_tiles = [
                    pso_pool.tile([P, D], f32, tag=f"pso{i}", name=f"pso{i}")
                    for i in range(BTT)
                ]
                srow_ps = psr_pool.tile([1, CHUNK], f32, tag="psrow")
                w_tiles = [None] * NG

                def emit_g1(g, xt=xt, c0=c0, w_tiles=w_tiles):
                    q_g = q_pool.tile([P, GRP, CHUNK], bf16, tag="q")
                    for jj in range(GRP):
                        j = g * GRP + jj
                        psq = psq_pool.tile([P, CHUNK], f32, tag="psq")
                        for k in range(KT):
                            nc.tensor.matmul(
                                psq,
                                posT_tiles[k][:, j * P : (j + 1) * P],
                                xt[:, k, :],
                                start=(k == 0),
                                stop=(k == KT - 1),
                            )
                        # q = psq + x2[r]
                        nc.vector.tensor_tensor(
                            q_g[:, jj, :],
                            psq,
                            x2b_sb[:, c0 : c0 + CHUNK],
                            Alu.add,
                        )
                    # d = sqrt(q + p2[n]) (batched per func to avoid ACT
                    # table swaps), then w = exp(-d / eff_t)
                    for jj in range(GRP):
                        j = g * GRP + jj
                        nc.scalar.activation(
                            out=q_g[:, jj, :],
                            in_=q_g[:, jj, :],
                            func=Act.Sqrt,
                            bias=p2_sb[:, j : j + 1],
                        )
                    w_g = w_pool.tile([P, GRP, CHUNK], bf16, tag="w")
                    for jj in range(GRP):
                        j = g * GRP + jj
                        nc.scalar.activation(
                            out=w_g[:, jj, :],
                            in_=q_g[:, jj, :],
                            func=Act.Exp,
                            scale=nit_sb[:, j : j + 1],
                        )
                    w_tiles[g] = w_g

                def emit_g2(g, pso_tiles=pso_tiles, srow_ps=srow_ps, w_tiles=w_tiles):
                    w_g = w_tiles[g]
                    for jj in range(GRP):
                        j = g * GRP + jj
                        first = j == 0
                        last = j == NT - 1
                        # row sums: s_row[1, r] += ones.T @ w_g[:, jj, :]
                        nc.tensor.matmul(
                            srow_ps,
                            ones_sb,
                            w_g[:, jj, :],
                            start=first,
                            stop=last,
                        )
                        for i in range(BTT):
                            lhsT = w_g[:, jj, i * P : (i + 1) * P]
                            nc.tensor.matmul(
                                pso_tiles[i],
                                lhsT,
                                v_sb[:, j, :],
                                start=first,
                                stop=last,
                            )

                def make_drain(c0=c0, pso_tiles=pso_tiles, srow_ps=srow_ps):
                    def drain():
                        # normalize: out_i = pso_i * (1 / (s_i + 1e-8))
                        s_sb = sr_pool.tile([1, CHUNK], f32, tag="s", name="s_sb")
                        nc.vector.tensor_tensor(s_sb, srow_ps, eps_sb, Alu.add)
                        # transpose (s + eps) [1, CHUNK] -> [P, BTT] via K=1
                        # matmuls (single group: start clears the bank, later
                        # MMs overwrite their own fresh columns)
                        rpt_ps = rpt_pool.tile([P, BTT], f32, tag="rpt", name="rpt")
                        for i in range(BTT):
                            nc.tensor.matmul(
                                rpt_ps[:, i : i + 1],
                                s_sb[0:1, i * P : (i + 1) * P],
                                one1_sb,
                                start=(i == 0),
                                stop=(i == BTT - 1),
                            )
                        # reciprocal on 128 partitions instead of on [1, CHUNK]
                        r_sb = sr_pool.tile([P, BTT], f32, tag="r", name="r_sb")
                        nc.vector.reciprocal(out=r_sb, in_=rpt_ps)
                        for i in range(BTT):
                            o_sb = o_pool.tile([P, D], f32, tag="o", name="o_sb")
                            nc.vector.tensor_tensor(
                                o_sb,
                                pso_tiles[i],
                                r_sb[:, i : i + 1].to_broadcast([P, D]),
                                Alu.mult,
                            )
                            nc.gpsimd.dma_start(
                                out=out[c0 + i * P : c0 + (i + 1) * P, :], in_=o_sb
                            )
                    return drain

                for g in range(NG):
                    emit_g1(g)
                    if g == 0 and pending_drain[0] is not None:
                        # drain the previous chunk while this chunk's first
                        # GEMM1 group keeps the PE busy
                        pending_drain[0]()
                        pending_drain[0] = None
                    if g >= 1:
                        emit_g2(g - 1)
                emit_g2(NG - 1)
                pending_drain[0] = make_drain()
            pending_drain[0]()
    nc.finalize()
    return nc


def prepare_in_maps(x, positions, values, temperature, n_cores=N_CORES):
    bf16 = ml_dtypes.bfloat16
    x = np.asarray(x, np.float32)
    positions = np.asarray(positions, np.float32)
    values = np.asarray(values, np.float32)
    temperature = np.asarray(temperature, np.float32)

    B, T, D = x.shape
    N = positions.shape[0]
    xf = x.reshape(-1, D)
    R = xf.shape[0] // n_cores

    # attention scale with ages=0: 0.05 + 0.95 * (1 - exp(0)) = 0.05
    eff_t = (np.abs(temperature) + 0.1) * np.float32(0.05)
    nit_full = (-1.0 / eff_t).astype(np.float32)        # [N]
    p2_full = (positions * positions).sum(1).astype(np.float32)  # [N]
    NT = N // P
    p2_pt = np.ascontiguousarray(p2_full.reshape(NT, P).T)
    nit_pt = np.ascontiguousarray(nit_full.reshape(NT, P).T)
    posTm2 = np.ascontiguousarray((-2.0 * positions).T).astype(bf16)
    v_bf = np.ascontiguousarray(values).astype(bf16)

    maps = []
    for ci in range(n_cores):
        xc = xf[ci * R : (ci + 1) * R]
        x2c = (xc * xc).sum(1, dtype=np.float32)
        aux = np.empty((P, R + 2 * NT), np.float32)
        aux[:, :R] = x2c[None, :]
        aux[:, R : R + NT] = p2_pt
        aux[:, R + NT : R + 2 * NT] = nit_pt
        maps.append(
            dict(
                xT=np.ascontiguousarray(xc.T).astype(bf16),
                posTm2=posTm2,
                v=v_bf,
                aux=aux,
            )
        )
    return maps


_prog_cache = {}


def get_program():
    if "nc" not in _prog_cache:
        _prog_cache["nc"] = build_program()
    return _prog_cache["nc"]


def kernel(x, positions, values, temperature):
    from concourse.bass_utils import run_bass_kernel_spmd

    maps = prepare_in_maps(x, positions, values, temperature)
    nc = get_program()
    res = run_bass_kernel_spmd(nc, maps, list(range(N_CORES)))
    B, T, D = np.asarray(x).shape
    out = np.concatenate(
        [np.asarray(res.results[i]["out"]) for i in range(N_CORES)], axis=0
    )
    return np.ascontiguousarray(out.reshape(B, T, D)).astype(np.float32)
